# revision 1
# baseline (speedup 1.0000x reference)
"""ClusterNorm1d kernel for Trainium2 (Bass/Tile), 8-core data parallel.

out[b,d,k] = sum_e Std_inv[k,d,e] * (x[b,e,k] - mu[e,k])

Strategy:
  - Shard batch B=8192 across 8 cores (1024 rows each); replicate the small
    mu / Std_inv derived buffers on every core.
  - Per core, process batch tiles of 128 rows. Clusters are processed in
    PAIRS (k = j, j+64) so the contraction over e uses the full 128-row PE
    array: the pair's two 64x64 matrices are packed block-diagonally (in an
    interleaved row/col order c = 2e+p, n = 2d+p) into a 128x128 weight
    panel. The pair choice (j, j+64) makes the x slice for one pair a
    SINGLE strided free dim (offset j, stride 64, count 128), which the
    walrus matmul verifier requires for the stationary operand.
    Per pair:
       1. PE transpose of x slice [b=128, c=128] -> PSUM [c, b]
       2. DVE copy PSUM->SBUF fused with per-partition mu subtraction
       3. PE matmul: lhsT = (x-mu)^T [c, b], rhs = W_j [c, n] -> PSUM [b, n]
       4. ACT copy PSUM->SBUF output staging at stride-64 offsets so the
          final DMA out is fully contiguous.
"""

import numpy as np

B, D, K = 8192, 64, 128
N_CORES = 8
B_SHARD = B // N_CORES  # 1024
P = 128                 # SBUF partitions = batch tile size
NPAIR = K // 2          # 64 cluster pairs: (j, j+64)

_cache = {}


def _build_nc(b_shard):
    import concourse.tile as tile
    from concourse import bacc, mybir
    from concourse.masks import make_identity

    f32 = mybir.dt.float32
    nc = bacc.Bacc("TRN2", target_bir_lowering=False)

    x_d = nc.dram_tensor("x", [b_shard, D * K], f32, kind="ExternalInput")
    w_d = nc.dram_tensor("w", [2 * D, NPAIR, 2 * D], f32, kind="ExternalInput")
    bias_d = nc.dram_tensor("bias", [P, D * K], f32, kind="ExternalInput")
    o_d = nc.dram_tensor("out", [b_shard, D * K], f32, kind="ExternalOutput")

    ntiles = b_shard // P
    NG = NPAIR // 4  # 16 groups of 4 pairs; one PSUM bank per group

    with tile.TileContext(nc) as tc:
        with (
            tc.tile_pool(name="consts", bufs=1) as consts,
            tc.tile_pool(name="xin", bufs=2) as xin,
            tc.tile_pool(name="xt", bufs=3) as xtp,
            tc.tile_pool(name="oout", bufs=2) as oout,
            tc.tile_pool(name="psT", bufs=3, space="PSUM") as psT,
            tc.tile_pool(name="psO", bufs=3, space="PSUM") as psO,
        ):
            ident = consts.tile([P, P], f32)
            make_identity(nc, ident)
            w_sb = consts.tile([2 * D, NPAIR, 2 * D], f32)
            nc.sync.dma_start(out=w_sb, in_=w_d[:])
            # bias replicated across partitions, grouped (g, q, d, p) order
            bias_sb = consts.tile([P, D * K], f32)
            nc.sync.dma_start(out=bias_sb, in_=bias_d[:])

            # Engine warm-ups: observe const semaphores once each.
            warm_ps = psT.tile([P, 4, P], f32, tag="psT_bank")
            nc.tensor.transpose(warm_ps[:, 0, :], ident, ident)
            nc.tensor.matmul(warm_ps[:, 1, :], lhsT=ident, rhs=w_sb[:, 0, :])
            scratch = consts.tile([P, 1], f32)
            nc.vector.tensor_copy(scratch, bias_sb[:, 0:1])

            for t in range(ntiles):
                x_t = xin.tile([P, D * K], f32, tag="x_t")
                nc.sync.dma_start(out=x_t, in_=x_d[t * P:(t + 1) * P])
                # [:, j, :] = offset j, stride 64, count 128 (pair k=j, j+64)
                x_w = x_t.rearrange("b (t s) -> b s t", s=NPAIR)
                o_t = oout.tile([P, D * K], f32)
                # staging f = 128d + 64p + 4g + q  ->  [b, g, q, d, p]
                o_v = o_t.rearrange("b (d p g q) -> b g q d p", p=2, g=NG, q=4)
                # absorb the out-buffer release wait cheaply
                nc.vector.tensor_copy(out=o_t[:, 0:1], in_=bias_sb[:, 0:1])
                for g in range(NG):
                    psb = psT.tile([P, 4, P], f32, tag="psT_bank")
                    for q in range(4):
                        nc.tensor.transpose(
                            psb[:, q, :], x_w[:, 4 * g + q, :], ident)
                    xt_s = xtp.tile([P, 4, P], f32)
                    nc.scalar.copy(out=xt_s, in_=psb)
                    osb = psO.tile([P, 4, P], f32)
                    for q in range(4):
                        nc.tensor.matmul(
                            osb[:, q, :], lhsT=xt_s[:, q, :],
                            rhs=w_sb[:, 4 * g + q, :])
                    nc.vector.tensor_sub(
                        o_v[:, g],
                        osb.rearrange("b q (d p) -> b q d p", p=2),
                        bias_sb[:, 512 * g:512 * (g + 1)].rearrange(
                            "b (q d p) -> b q d p", q=4, p=2),
                    )
                nc.sync.dma_start(out=o_d[t * P:(t + 1) * P], in_=o_t)

    nc.compile()
    return nc


def _host_prep(mu_track, Std_inv_track):
    """Pack W [2D, NPAIR, 2D] with c=2e+p, n=2d+p, pair j = (k=j, k=j+64),
    and the replicated bias panel in grouped (g, q, d, p) order."""
    W = np.zeros((2 * D, NPAIR, 2 * D), dtype=np.float32)
    W6 = W.reshape(D, 2, NPAIR, D, 2)                 # [e, p, j, d, p']
    S_r = np.ascontiguousarray(Std_inv_track, dtype=np.float32).reshape(
        2, NPAIR, D, D)                               # [p, j, d, e]
    W6[:, 0, :, :, 0] = S_r[0].transpose(2, 0, 1)     # [e, j, d]
    W6[:, 1, :, :, 1] = S_r[1].transpose(2, 0, 1)
    S = np.ascontiguousarray(Std_inv_track, dtype=np.float32)
    mu = np.ascontiguousarray(mu_track, dtype=np.float32)
    bias_dk = np.einsum("kde,ek->dk", S, mu)          # [d, k], k = 64p+4g+q
    bias_g = bias_dk.reshape(D, 2, NPAIR // 4, 4).transpose(
        2, 3, 0, 1).reshape(D * K)                    # (g, q, d, p)
    bias = np.broadcast_to(bias_g, (P, D * K)).copy()
    return W, bias


def kernel(x, mu_track, Std_inv_track):
    from concourse.bass_utils import run_bass_kernel_spmd

    x = np.ascontiguousarray(x, dtype=np.float32).reshape(B, D * K)
    W, bias = _host_prep(mu_track, Std_inv_track)

    if "nc" not in _cache:
        _cache["nc"] = _build_nc(B_SHARD)
    nc = _cache["nc"]

    in_maps = []
    for i in range(N_CORES):
        in_maps.append({
            "x": x[i * B_SHARD:(i + 1) * B_SHARD],
            "w": W,
            "bias": bias,
        })
    res = run_bass_kernel_spmd(nc, in_maps, core_ids=list(range(N_CORES)))
    out = np.concatenate([r["out"] for r in res.results], axis=0)
    return out.reshape(B, D, K)



# revision 2
# speedup vs baseline: 2.0582x; 2.0582x over previous
"""ClusterNorm1d kernel for Trainium2 (Bass/Tile), 8-core data parallel.

out[b,d,k] = sum_e Std_inv[k,d,e] * (x[b,e,k] - mu[e,k])

Strategy (v2, bf16 / transpose-free):
  - Shard batch B=8192 across 8 cores (1024 rows each).
  - Host packs x pre-transposed and pair-interleaved in bf16:
      xt[c, j, b] = x[b, e, j + 64*p]   with c = 2e + p
    so clusters (j, j+64) share one 128-deep contraction. Weights are the
    same block-diagonal pair panels as before:
      W[c=2e+pc, j, n=2d+pd] = S[j+64*pd, d, e] * (pc == pd)
  - Device work per pair j is then a single stationary-weight matmul
      psum[n, b] = sum_c W[c, j, n] * xt[c, j, b]
    (no on-device transpose at all), followed by a PSUM->SBUF drain fused
    with the -S@mu bias (per-partition scalar), alternating ACT / DVE.
  - Output returns transposed+packed [n', j, b] in bf16; the host unpacks
    to [B, D, K] f32. fp32 matmul costs 4 PE cycles/row vs 1 for bf16, so
    the bf16 datapath also removes the PE bottleneck (rel err ~1e-2 budget).
  - DMA: input stream on the SP HWDGE queue, output stream on the ACT
    HWDGE queue, 4 pairs (1 MiB) per transfer, 8 KiB contiguous per
    partition row.
"""

import numpy as np

B, D, K = 8192, 64, 128
N_CORES = 8
B_SHARD = B // N_CORES  # 1024
P = 128                 # SBUF partitions
NPAIR = K // 2          # 64 cluster pairs: (j, j+64)
GP = 4                  # pairs per DMA chunk
SEG = 512               # matmul moving free-dim per PSUM bank

_cache = {}


def _build_nc(b_shard):
    import concourse.tile as tile
    from concourse import bacc, mybir

    f32 = mybir.dt.float32
    bf16 = mybir.dt.bfloat16
    nc = bacc.Bacc("TRN2", target_bir_lowering=False)

    xt_d = nc.dram_tensor("xt", [P, NPAIR, b_shard], bf16, kind="ExternalInput")
    w_d = nc.dram_tensor("w", [P, NPAIR, P], bf16, kind="ExternalInput")
    nb_d = nc.dram_tensor("nbias", [P, NPAIR], f32, kind="ExternalInput")
    o_d = nc.dram_tensor("out", [P, NPAIR, b_shard], bf16, kind="ExternalOutput")

    seg = min(SEG, b_shard)
    nseg = b_shard // seg
    nchunk = NPAIR // GP

    with tile.TileContext(nc) as tc:
        with (
            tc.tile_pool(name="consts", bufs=1) as consts,
            tc.tile_pool(name="xin", bufs=3) as xin,
            tc.tile_pool(name="oout", bufs=3) as oout,
            tc.tile_pool(name="ps", bufs=6, space="PSUM") as psp,
        ):
            w_sb = consts.tile([P, NPAIR, P], bf16)
            nb_sb = consts.tile([P, NPAIR], f32)
            # consts ride the ACT queue: the SP queue starts streaming x
            # immediately and the out stream doesn't exist yet.
            nc.scalar.dma_start(out=w_sb, in_=w_d[:])
            nc.scalar.dma_start(out=nb_sb, in_=nb_d[:])

            # Engine warm-ups: observe const semaphores once each.
            warm_ps = psp.tile([P, seg], f32, tag="ps")
            nc.tensor.matmul(warm_ps[:, 0:P], lhsT=w_sb[:, 0, :],
                             rhs=w_sb[:, 0, :])
            scratch = consts.tile([P, 2], f32)
            nc.scalar.copy(out=scratch[:, 0:1], in_=nb_sb[:, 0:1])
            nc.vector.tensor_copy(out=scratch[:, 1:2], in_=nb_sb[:, 0:1])

            for ch in range(nchunk):
                xt = xin.tile([P, GP, b_shard], bf16, tag="xt")
                nc.sync.dma_start(out=xt, in_=xt_d[:, ch * GP:(ch + 1) * GP, :])
                o_sb = oout.tile([P, GP, b_shard], bf16, tag="o")
                for jj in range(GP):
                    j = ch * GP + jj
                    nbj = nb_sb[:, j:j + 1]
                    for h in range(nseg):
                        ps = psp.tile([P, seg], f32, tag="ps")
                        nc.tensor.matmul(
                            ps, lhsT=w_sb[:, j, :],
                            rhs=xt[:, jj, h * seg:(h + 1) * seg])
                        dst = o_sb[:, jj, h * seg:(h + 1) * seg]
                        if (2 * jj + h) % 2 == 0:
                            nc.scalar.add(dst, ps, nbj)
                        else:
                            nc.vector.tensor_scalar_add(dst, ps, nbj)
                nc.scalar.dma_start(out=o_d[:, ch * GP:(ch + 1) * GP, :],
                                    in_=o_sb)

    nc.compile()
    return nc


def _host_prep(mu_track, Std_inv_track):
    """Block-diagonal pair weights W[c=2e+pc, j, n=2d+pd] (bf16) and the
    negated per-partition bias nbias[n'=2d+p, j] = -(S@mu)[d, j+64p] (f32)."""
    import ml_dtypes

    S = np.ascontiguousarray(Std_inv_track, dtype=np.float32)
    mu = np.ascontiguousarray(mu_track, dtype=np.float32)

    W = np.zeros((2 * D, NPAIR, 2 * D), dtype=np.float32)
    W6 = W.reshape(D, 2, NPAIR, D, 2)                 # [e, pc, j, d, pd]
    S_r = S.reshape(2, NPAIR, D, D)                   # [pk, j, d, e]
    W6[:, 0, :, :, 0] = S_r[0].transpose(2, 0, 1)     # [e, j, d]
    W6[:, 1, :, :, 1] = S_r[1].transpose(2, 0, 1)

    bias_dk = np.einsum("kde,ek->dk", S, mu)          # [d, k], k = 64p + j
    nbias = -bias_dk.reshape(D, 2, NPAIR).reshape(2 * D, NPAIR)  # [n'=2d+p, j]
    return W.astype(ml_dtypes.bfloat16), np.ascontiguousarray(nbias)


def _pack_x(x, n_cores, b_shard):
    """x [n_cores*b_shard, D, K] f32 -> xt [n_cores, 128, NPAIR, b_shard] bf16
    with xt[core, 2e+p, j, b] = x[core*b_shard + b, e, j + 64p]."""
    import ml_dtypes

    xb = np.ascontiguousarray(x, dtype=np.float32).astype(ml_dtypes.bfloat16)
    xp = xb.reshape(n_cores, b_shard, D, 2, NPAIR)    # [core, b, e, p, j]
    xt = xp.transpose(0, 2, 3, 4, 1)                  # [core, e, p, j, b]
    return np.ascontiguousarray(xt).reshape(n_cores, P, NPAIR, b_shard)


def _unpack_out(oT, n_cores, b_shard):
    """oT [n_cores, 128, NPAIR, b_shard] bf16 -> out [n_cores*b_shard, D, K]
    f32 with out[b, d, j+64p] = oT[core, 2d+p, j, b]."""
    ov = oT.reshape(n_cores, D, 2, NPAIR, b_shard)    # [core, d, p, j, b]
    out = ov.transpose(0, 4, 1, 2, 3)                 # [core, b, d, p, j]
    return np.ascontiguousarray(out).reshape(
        n_cores * b_shard, D, K).astype(np.float32)


def kernel(x, mu_track, Std_inv_track):
    from concourse.bass_utils import run_bass_kernel_spmd

    xt = _pack_x(x, N_CORES, B_SHARD)
    W, nbias = _host_prep(mu_track, Std_inv_track)

    if "nc" not in _cache:
        _cache["nc"] = _build_nc(B_SHARD)
    nc = _cache["nc"]

    in_maps = []
    for i in range(N_CORES):
        in_maps.append({"xt": xt[i], "w": W, "nbias": nbias})
    res = run_bass_kernel_spmd(nc, in_maps, core_ids=list(range(N_CORES)))
    oT = np.stack([r["out"] for r in res.results], axis=0)
    return _unpack_out(oT, N_CORES, B_SHARD)


# revision 4
# speedup vs baseline: 2.2780x; 1.1068x over previous
"""ClusterNorm1d kernel for Trainium2 (Bass/Tile), 8-core data parallel.

out[b,d,k] = sum_e Std_inv[k,d,e] * (x[b,e,k] - mu[e,k])

Strategy (v2, bf16 / transpose-free):
  - Shard batch B=8192 across 8 cores (1024 rows each).
  - Host packs x pre-transposed and pair-interleaved in bf16:
      xt[c, j, b] = x[b, e, j + 64*p]   with c = 2e + p
    so clusters (j, j+64) share one 128-deep contraction. Weights are the
    same block-diagonal pair panels as before:
      W[c=2e+pc, j, n=2d+pd] = S[j+64*pd, d, e] * (pc == pd)
  - Device work per pair j is then a single stationary-weight matmul
      psum[n, b] = sum_c W[c, j, n] * xt[c, j, b]
    (no on-device transpose at all), followed by a PSUM->SBUF drain fused
    with the -S@mu bias (per-partition scalar), alternating ACT / DVE.
  - Output returns transposed+packed [n', j, b] in bf16; the host unpacks
    to [B, D, K] f32. fp32 matmul costs 4 PE cycles/row vs 1 for bf16, so
    the bf16 datapath also removes the PE bottleneck (rel err ~1e-2 budget).
  - DMA: input stream on the SP HWDGE queue, output stream on the ACT
    HWDGE queue, 4 pairs (1 MiB) per transfer, 8 KiB contiguous per
    partition row.
"""

import numpy as np

B, D, K = 8192, 64, 128
N_CORES = 8
B_SHARD = B // N_CORES  # 1024
P = 128                 # SBUF partitions
NPAIR = K // 2          # 64 cluster pairs: (j, j+64)
SEG = 512               # matmul moving free-dim per PSUM bank

# DMA chunking (pairs per transfer). Packet size per partition row is
# 2*b_shard*csize bytes; bigger packets amortize the ~100ns/packet engine
# overhead, but the head (input) and tail (output) want small chunks so the
# pipeline starts/drains early.
IN_CHUNKS = [2, 2, 4] + [8] * 7
OUT_CHUNKS = [8] * 7 + [4, 2, 2]
W_CHUNKS = 4            # w panel DMA'd in 4 pair-range chunks

_cache = {}


def _bounds(chunks):
    out, s = [], 0
    for c in chunks:
        out.append((s, c))
        s += c
    return out


def _build_nc(b_shard):
    import concourse.tile as tile
    from concourse import bacc, mybir

    f32 = mybir.dt.float32
    bf16 = mybir.dt.bfloat16
    nc = bacc.Bacc("TRN2", target_bir_lowering=False)

    xt_d = nc.dram_tensor("xt", [P, NPAIR, b_shard], bf16, kind="ExternalInput")
    w_d = nc.dram_tensor("w", [P, NPAIR, P], bf16, kind="ExternalInput")
    nb_d = nc.dram_tensor("nbias", [P, NPAIR], f32, kind="ExternalInput")
    o_d = nc.dram_tensor("out", [P, NPAIR, b_shard], bf16, kind="ExternalOutput")

    seg = min(SEG, b_shard)
    nseg = b_shard // seg
    in_bounds = _bounds(IN_CHUNKS)
    out_bounds = _bounds(OUT_CHUNKS)

    with tile.TileContext(nc) as tc:
        with (
            tc.tile_pool(name="consts", bufs=1) as consts,
            tc.tile_pool(name="xin", bufs=3) as xin,
            tc.tile_pool(name="oout", bufs=3) as oout,
            tc.tile_pool(name="ps", bufs=6, space="PSUM") as psp,
        ):
            w_sb = consts.tile([P, NPAIR, P], bf16)
            nb_sb = consts.tile([P, NPAIR], f32)
            # Consts ride the ACT queue (the out stream is idle at t=0; the
            # SP queue starts streaming x immediately). w is chunked so the
            # first matmul only waits for the first pair-range.
            wc = NPAIR // W_CHUNKS
            nc.scalar.dma_start(out=w_sb[:, 0:wc, :], in_=w_d[:, 0:wc, :])
            nc.scalar.dma_start(out=nb_sb, in_=nb_d[:])
            for q in range(1, W_CHUNKS):
                nc.scalar.dma_start(out=w_sb[:, q * wc:(q + 1) * wc, :],
                                    in_=w_d[:, q * wc:(q + 1) * wc, :])

            # Engine warm-ups: observe const semaphores once each.
            warm_ps = psp.tile([P, seg], f32, tag="ps")
            nc.tensor.matmul(warm_ps[:, 0:P], lhsT=w_sb[:, 0, :],
                             rhs=w_sb[:, 0, :])
            scratch = consts.tile([P, 2], f32)
            nc.scalar.copy(out=scratch[:, 0:1], in_=nb_sb[:, 0:1])
            nc.vector.tensor_copy(out=scratch[:, 1:2], in_=nb_sb[:, 0:1])

            in_it = iter(in_bounds)
            out_it = iter(out_bounds)
            xt = o_sb = None
            in_s = in_n = out_s = out_n = 0
            for j in range(NPAIR):
                if xt is None or j >= in_s + in_n:
                    in_s, in_n = next(in_it)
                    xt = xin.tile([P, in_n, b_shard], bf16, tag="xt")
                    nc.sync.dma_start(
                        out=xt, in_=xt_d[:, in_s:in_s + in_n, :])
                if o_sb is None or j >= out_s + out_n:
                    out_s, out_n = next(out_it)
                    o_sb = oout.tile([P, out_n, b_shard], bf16, tag="o")
                nbj = nb_sb[:, j:j + 1]
                for h in range(nseg):
                    ps = psp.tile([P, seg], f32, tag="ps")
                    nc.tensor.matmul(
                        ps, lhsT=w_sb[:, j, :],
                        rhs=xt[:, j - in_s, h * seg:(h + 1) * seg])
                    dst = o_sb[:, j - out_s, h * seg:(h + 1) * seg]
                    if (2 * j + h) % 2 == 0:
                        nc.scalar.add(dst, ps, nbj)
                    else:
                        nc.vector.tensor_scalar_add(dst, ps, nbj)
                if j == out_s + out_n - 1:
                    nc.scalar.dma_start(
                        out=o_d[:, out_s:out_s + out_n, :], in_=o_sb)

    nc.compile()
    return nc


def _host_prep(mu_track, Std_inv_track):
    """Block-diagonal pair weights W[c=2e+pc, j, n=2d+pd] (bf16) and the
    negated per-partition bias nbias[n'=2d+p, j] = -(S@mu)[d, j+64p] (f32)."""
    import ml_dtypes

    S = np.ascontiguousarray(Std_inv_track, dtype=np.float32)
    mu = np.ascontiguousarray(mu_track, dtype=np.float32)

    W = np.zeros((2 * D, NPAIR, 2 * D), dtype=np.float32)
    W6 = W.reshape(D, 2, NPAIR, D, 2)                 # [e, pc, j, d, pd]
    S_r = S.reshape(2, NPAIR, D, D)                   # [pk, j, d, e]
    W6[:, 0, :, :, 0] = S_r[0].transpose(2, 0, 1)     # [e, j, d]
    W6[:, 1, :, :, 1] = S_r[1].transpose(2, 0, 1)

    bias_dk = np.einsum("kde,ek->dk", S, mu)          # [d, k], k = 64p + j
    nbias = -bias_dk.reshape(D, 2, NPAIR).reshape(2 * D, NPAIR)  # [n'=2d+p, j]
    return W.astype(ml_dtypes.bfloat16), np.ascontiguousarray(nbias)


def _pack_x(x, n_cores, b_shard):
    """x [n_cores*b_shard, D, K] f32 -> xt [n_cores, 128, NPAIR, b_shard] bf16
    with xt[core, 2e+p, j, b] = x[core*b_shard + b, e, j + 64p]."""
    import ml_dtypes

    xb = np.ascontiguousarray(x, dtype=np.float32).astype(ml_dtypes.bfloat16)
    xp = xb.reshape(n_cores, b_shard, D, 2, NPAIR)    # [core, b, e, p, j]
    xt = xp.transpose(0, 2, 3, 4, 1)                  # [core, e, p, j, b]
    return np.ascontiguousarray(xt).reshape(n_cores, P, NPAIR, b_shard)


def _unpack_out(oT, n_cores, b_shard):
    """oT [n_cores, 128, NPAIR, b_shard] bf16 -> out [n_cores*b_shard, D, K]
    f32 with out[b, d, j+64p] = oT[core, 2d+p, j, b]."""
    ov = oT.reshape(n_cores, D, 2, NPAIR, b_shard)    # [core, d, p, j, b]
    out = ov.transpose(0, 4, 1, 2, 3)                 # [core, b, d, p, j]
    return np.ascontiguousarray(out).reshape(
        n_cores * b_shard, D, K).astype(np.float32)


def kernel(x, mu_track, Std_inv_track):
    from concourse.bass_utils import run_bass_kernel_spmd

    xt = _pack_x(x, N_CORES, B_SHARD)
    W, nbias = _host_prep(mu_track, Std_inv_track)

    if "nc" not in _cache:
        _cache["nc"] = _build_nc(B_SHARD)
    nc = _cache["nc"]

    in_maps = []
    for i in range(N_CORES):
        in_maps.append({"xt": xt[i], "w": W, "nbias": nbias})
    res = run_bass_kernel_spmd(nc, in_maps, core_ids=list(range(N_CORES)))
    oT = np.stack([r["out"] for r in res.results], axis=0)
    return _unpack_out(oT, N_CORES, B_SHARD)
